# revision 3
# baseline (speedup 1.0000x reference)
"""Trainium2 Bass kernel for nn_MiddleLayerEncoder (gnn_message_passing), v2.

Cluster-sharded across 8 cores (512 whole clusters per core, all segment
maxes core-local, no collectives). Host prep sorts points by cluster, pads
each cluster to a canonical per-rank size (identical across cores -> one
SPMD program) and bakes all segment boundaries into the instruction stream.

Key design points (vs the v1 baseline this evolved from):
  - neigh_enc[cluster] gather fused into the layer-1 matmul via a
    per-1024-col-group one-hot (G=24 rows baked into encT); one lhsT
    [91,128] serves a whole group, rotating lhsT buffers carry W1ab
    preloaded once so only the 24 M rows are re-DMAed per group.
  - all PSUM evacuations are contiguous; layer-2's intra-cluster pairing
    comes from even/odd strided *moving* matmul operands (full speed on PE),
    pair-max via Act copy + DVE tensor_max (TT allows only one PSUM input).
  - segment maxes: single DVE reduce_max per size-class directly on the
    bf16 SBUF buffers (beats halving trees at measured TRN2 rates).
  - streaming: 4096-col chunks, encT split into 1024-col slab tiles so the
    chunk DMAs spread across many SDMA engines; small/M-row DMAs issued on
    the gpsimd software-DGE queue to keep the SP HWDGE ring clear.
  - evac work split across Act and DVE (L1_EVAC pattern), PSUM pooled as
    2x[128,512] (enc) + 3x[128,1024] (l1/l2) so PE can run ahead.
"""

import numpy as np
import ml_dtypes
from contextlib import ExitStack

import concourse.bass as bass
import concourse.bacc as bacc
import concourse.tile as tile
from concourse import mybir
from concourse.bass_utils import run_bass_kernel_spmd

BF16 = mybir.dt.bfloat16
F32 = mybir.dt.float32
NPBF16 = ml_dtypes.bfloat16

N_CORES = 8
N_PTS = 262144
N_CLUSTERS = 4096
G = 24            # one-hot rows (max clusters overlapping a 1024-col group)
K1 = 3 + 64 + G   # layer-1 contraction: pts(3) + feat(64) + onehot(G)
MINL = 36         # minimum padded points per cluster
CHUNK_COLS = 4096
GROUP = 1024      # l1 lhsT group width
TILE = 512
SLAB = 1024      # encT DMA slab width (distinct buffers spread DMA engines)

# which engine evacuates each of the 4 l1 psum tiles per group
L1_EVAC = ("v", "a", "a", "a")
BOTH_PSUM_TT = False  # illegal on TRN2: only one TT input may be PSUM


# ---------------------------------------------------------------- planning

def _plan(cluster):
    """Canonical SPMD layout shared by all cores."""
    counts = np.bincount(cluster, minlength=N_CLUSTERS)
    assert counts.min() >= 1, "empty cluster unsupported"
    order = np.argsort(-counts, kind="stable")  # cluster ids, size desc

    n_ranks = N_CLUSTERS // N_CORES
    cids = np.empty((N_CORES, n_ranks), dtype=np.int64)
    for i, cid in enumerate(order):
        rnd, pos = divmod(i, N_CORES)
        core = pos if rnd % 2 == 0 else N_CORES - 1 - pos
        cids[core, rnd] = cid

    sizes = counts[cids]                      # [cores, ranks]
    L = sizes.max(axis=0)                     # canonical per-rank size
    L = np.maximum((L + 3) // 4 * 4, MINL).astype(np.int64)

    col0 = np.concatenate([[0], np.cumsum(L)])  # rank -> start col
    S = int(col0[-1])

    # chunks: whole clusters, <= CHUNK_COLS cols, <= 128 clusters
    chunks = []  # (r0, r1, c0, cols)
    r0 = 0
    while r0 < n_ranks:
        r1 = r0
        while (
            r1 < n_ranks
            and (col0[r1 + 1] - col0[r0]) <= CHUNK_COLS
            and (r1 - r0) < 128
        ):
            r1 += 1
        chunks.append((r0, r1, int(col0[r0]), int(col0[r1] - col0[r0])))
        r0 = r1

    col_rank = np.repeat(np.arange(n_ranks), L)

    # l1 groups per chunk: (c_abs, cols, base_rank, n_ranks_in_group)
    groups = []
    for (r0, r1, c0, cc) in chunks:
        gl = []
        for t0 in range(0, cc, GROUP):
            tc = min(GROUP, cc - t0)
            base = int(col_rank[c0 + t0])
            last = int(col_rank[c0 + t0 + tc - 1])
            nrows = last - base + 1
            assert nrows <= G, f"group spans {nrows} clusters > G={G}"
            gl.append((c0 + t0, tc, base, nrows))
        groups.append(gl)

    # per-chunk size classes: runs of equal L within the chunk
    cls_of = []
    for (r0, r1, c0, cc) in chunks:
        cl = []
        i = r0
        while i < r1:
            j = i
            while j < r1 and L[j] == L[i]:
                j += 1
            cl.append((i, j - i, int(L[i])))
            i = j
        cls_of.append(cl)

    return dict(
        cids=cids, sizes=sizes, L=L, col0=col0, S=S, chunks=chunks,
        col_rank=col_rank, groups=groups, cls_of=cls_of, n_ranks=n_ranks,
    )


def _prep_core(k, plan, rel_points, features, sort_idx, bucket0):
    """Per-core input arrays (canonical layout, core-specific data)."""
    L, col0, S = plan["L"], plan["col0"], plan["S"]
    cids = plan["cids"][k]
    n_ranks = plan["n_ranks"]

    slot = np.empty(S, dtype=np.int64)
    for r in range(n_ranks):
        cid = cids[r]
        idx = sort_idx[bucket0[cid]: bucket0[cid + 1]]
        n = idx.shape[0]
        c0, c1 = col0[r], col0[r + 1]
        slot[c0: c0 + n] = idx
        if c1 - c0 > n:
            slot[c0 + n: c1] = idx[0]

    pts = rel_points[slot]          # [S, 3] f32
    feat = features[slot]           # [S, 64] f32

    # encT: [K1, S] = ptsT(3) + featT(64) + group-local onehot(G)
    encT = np.zeros((K1, S), dtype=NPBF16)
    encT[0:3] = pts.T.astype(NPBF16)
    encT[3:67] = feat.T.astype(NPBF16)
    col_rank = plan["col_rank"]
    oh_row = np.empty(S, dtype=np.int64)
    for gl in plan["groups"]:
        for (c0, tc, base, nrows) in gl:
            oh_row[c0: c0 + tc] = col_rank[c0: c0 + tc] - base
    encT[67 + oh_row, np.arange(S)] = NPBF16(1.0)

    pts4 = (
        pts.astype(NPBF16)
        .reshape(S // 4, 4, 3)
        .transpose(1, 2, 0)
        .reshape(12, S // 4)
    )
    return {"encT": encT, "pts4": np.ascontiguousarray(pts4)}


def _blockdiag(w, times):
    fi, fo = w.shape
    out = np.zeros((fi * times, fo * times), dtype=w.dtype)
    for i in range(times):
        out[i * fi:(i + 1) * fi, i * fo:(i + 1) * fo] = w
    return out


def _prep_weights(inp):
    bf = lambda a: np.ascontiguousarray(a.astype(NPBF16))
    f32c = lambda a: np.ascontiguousarray(a.reshape(-1, 1).astype(np.float32))
    W1 = inp["W1"]
    return {
        "enc1_lhsT": bf(_blockdiag(inp["enc_W1"], 4)),       # [12,128]
        "b_enc1_4": f32c(np.tile(inp["enc_b1"], 4)),          # [128,1]
        "enc2_lhsT": bf(_blockdiag(inp["enc_W2"], 2)),        # [64,128]
        "b_enc2": f32c(inp["enc_b2"]),                        # [64,1]
        "W1ab": bf(W1[0:67]),                                 # [67,128]
        "W1c": bf(W1[67:131]),                                # [64,128]
        "b1": f32c(inp["b1"]),
        "fcW2": bf(inp["W2"]),                                # [128,128]
        "b2": f32c(inp["b2"]),
        "G1": bf(inp["G1"]),
        "gb1": f32c(inp["gb1"]),
        "G2a": bf(inp["G2"][:, 0:128]),
        "G2b": bf(inp["G2"][:, 128:256]),
        "gb2a": f32c(inp["gb2"][0:128]),
        "gb2b": f32c(inp["gb2"][128:256]),
    }


# ---------------------------------------------------------------- program

def _build(plan, reps=1):
    S = plan["S"]
    n_ranks = plan["n_ranks"]
    nc = bacc.Bacc(None, target_bir_lowering=False, debug=True)

    encT_d = nc.dram_tensor("encT", [K1, S], BF16, kind="ExternalInput")
    pts4_d = nc.dram_tensor("pts4", [12, S // 4], BF16, kind="ExternalInput")
    wspec = [
        ("enc1_lhsT", [12, 128], BF16), ("b_enc1_4", [128, 1], F32),
        ("enc2_lhsT", [64, 128], BF16), ("b_enc2", [64, 1], F32),
        ("W1ab", [67, 128], BF16), ("W1c", [64, 128], BF16),
        ("b1", [128, 1], F32), ("fcW2", [128, 128], BF16),
        ("b2", [128, 1], F32), ("G1", [128, 128], BF16),
        ("gb1", [128, 1], F32), ("G2a", [128, 128], BF16),
        ("G2b", [128, 128], BF16), ("gb2a", [128, 1], F32),
        ("gb2b", [128, 1], F32),
    ]
    w_d = {n: nc.dram_tensor(n, sh, dt, kind="ExternalInput") for n, sh, dt in wspec}
    out_d = nc.dram_tensor("out", [256, 512], F32, kind="ExternalOutput")

    RELU = mybir.ActivationFunctionType.Relu
    COPY = mybir.ActivationFunctionType.Copy
    ADD = mybir.AluOpType.add
    MAX = mybir.AluOpType.max

    with tile.TileContext(nc) as tc, ExitStack() as ctx:
        consts = ctx.enter_context(tc.tile_pool(name="consts", bufs=1))
        glob = ctx.enter_context(tc.tile_pool(name="glob", bufs=1))
        stream = ctx.enter_context(tc.tile_pool(name="stream", bufs=3))
        stream2 = ctx.enter_context(tc.tile_pool(name="stream2", bufs=2))
        small = ctx.enter_context(tc.tile_pool(name="small", bufs=4))
        ps_e = ctx.enter_context(tc.tile_pool(name="ps_e", bufs=2, space="PSUM"))
        ps_l = ctx.enter_context(tc.tile_pool(name="ps_l", bufs=3, space="PSUM"))

        w_sb = {}
        for n, sh, dt in wspec:
            t = consts.tile(sh, dt, tag=f"w_{n}")
            nc.sync.dma_start(out=t[:], in_=w_d[n][:])
            w_sb[n] = t
        # enc2 weights also staged at partitions 64-127 (lhsT/rhs must share
        # base_partition; the B-half rhs lives there)
        enc2_hi = consts.tile([128, 128], BF16, tag="w_enc2_hi")
        nc.sync.dma_start(out=enc2_hi[64:128, :], in_=w_d["enc2_lhsT"][:])

        # rotating l1 lhsT buffers: W1ab rows preloaded once, M rows per group
        N_LT = 4
        lhts = []
        for i in range(N_LT):
            lt = consts.tile([K1, 128], BF16, tag=f"lt{i}")
            nc.vector.memset(lt[64:K1, :], 0.0)
            nc.sync.dma_start(out=lt[0:67, :], in_=w_d["W1ab"][:])
            lhts.append(lt)

        Cbuf = glob.tile([128, S // 4], BF16, tag="Cbuf")
        pre_neigh = glob.tile([128, n_ranks], BF16, tag="pre_neigh")
        neighT = glob.tile([64, n_ranks], BF16, tag="neighT")
        T2buf = glob.tile([128, n_ranks], BF16, tag="T2buf")
        gT = glob.tile([128, n_ranks], BF16, tag="gT")

        n_chunks = len(plan["chunks"])
        g_counter = [0]

        def enc_stage(k):
            (r0, r1, c0, cc) = plan["chunks"][k]
            q0, qc = c0 // 4, cc // 4
            pts4_t = stream.tile([12, GROUP], BF16, tag="pts4_t", bufs=2)
            n_slab = (cc + SLAB - 1) // SLAB
            encT_t = [stream.tile([K1, SLAB], BF16, tag=f"encT{i}",
                                  name=f"encT{i}") for i in range(n_slab)]
            nc.gpsimd.dma_start(out=pts4_t[:, :qc], in_=pts4_d[:, q0:q0 + qc])
            for i in range(n_slab):
                a = i * SLAB
                b = min(a + SLAB, cc)
                nc.sync.dma_start(out=encT_t[i][:, :b - a],
                                  in_=encT_d[:, c0 + a:c0 + b])
            h1_t = stream.tile([128, GROUP], BF16, tag="h1_t", bufs=2)
            # enc1: 512-col matmuls, 512-wide evacs
            for t0 in range(0, qc, TILE):
                tn = min(TILE, qc - t0)
                p1 = ps_e.tile([128, TILE], F32, tag="pse")
                nc.tensor.matmul(p1[:, :tn], w_sb["enc1_lhsT"][:],
                                 pts4_t[:, t0:t0 + tn], start=True, stop=True)
                nc.scalar.activation(h1_t[:, t0:t0 + tn], p1[:, :tn], RELU,
                                     bias=w_sb["b_enc1_4"][:], scale=1.0)
            # enc2: A/B pair in two 512 psum tiles per 512 cols
            for t0 in range(0, qc, TILE):
                tn = min(TILE, qc - t0)
                pA = ps_e.tile([128, TILE], F32, tag="pse")
                pB = ps_e.tile([128, TILE], F32, tag="pse")
                nc.tensor.matmul(pA[:, :tn], w_sb["enc2_lhsT"][:],
                                 h1_t[0:64, t0:t0 + tn], start=True, stop=True)
                nc.tensor.matmul(pB[:, :tn], enc2_hi[64:128, :],
                                 h1_t[64:128, t0:t0 + tn], start=True, stop=True)
                tA = small.tile([128, TILE], BF16, tag="tA", bufs=4)
                nc.scalar.activation(tA[:, :tn], pA[:, :tn], COPY)
                nc.vector.tensor_max(Cbuf[:, q0 + t0: q0 + t0 + tn],
                                     pB[:, :tn], tA[:, :tn])
            return encT_t

        def seg1_and_M(k):
            (r0, r1, c0, cc) = plan["chunks"][k]
            q0 = c0 // 4
            nk = r1 - r0
            for (ri, n, w) in plan["cls_of"][k]:
                o = q0 + (int(plan["col0"][ri]) - c0) // 4
                wq = w // 4
                nc.vector.reduce_max(
                    pre_neigh[:, ri:ri + n],
                    Cbuf[:, o: o + n * wq].rearrange("p (n w) -> p n w", w=wq),
                    axis=mybir.AxisListType.X)
            fold = small.tile([64, 128], BF16, tag="fold")
            nc.gpsimd.dma_start(out=fold[:, :nk], in_=pre_neigh[64:128, r0:r1])
            mx = small.tile([64, 128], BF16, tag="mx")
            nc.vector.tensor_max(mx[:, :nk], pre_neigh[0:64, r0:r1], fold[:, :nk])
            nc.scalar.activation(neighT[:, r0:r1], mx[:, :nk], RELU,
                                 bias=w_sb["b_enc2"][:], scale=1.0)
            pm = ps_l.tile([128, 1024], F32, tag="psl")
            nc.tensor.matmul(pm[:nk, :128], neighT[:, r0:r1], w_sb["W1c"][:],
                             start=True, stop=True)
            M_chunk = small.tile([128, 128], BF16, tag="Mchunk", bufs=2)
            nc.scalar.activation(M_chunk[:nk, :], pm[:nk, :128], COPY)
            return M_chunk

        def l_stage(k, M_chunk):
            (r0, r1, c0, cc) = plan["chunks"][k]
            encT_t = enc_tiles[k]
            e1_t = stream2.tile([128, CHUNK_COLS], BF16, tag="e1_t")
            ei = 0
            for (gc0, gcc, gbase, gnr) in plan["groups"][k]:
                lt = lhts[g_counter[0] % N_LT]
                g_counter[0] += 1
                nc.gpsimd.dma_start(out=lt[67:67 + gnr, :],
                                    in_=M_chunk[gbase - r0: gbase - r0 + gnr, :])
                loc = gc0 - c0
                # pairs of 512-col matmuls into [128,1024] psum; 1024-wide evac
                for t0 in range(0, gcc, 1024):
                    tn = min(1024, gcc - t0)
                    p = ps_l.tile([128, 1024], F32, tag="psl")
                    for u in range(0, tn, TILE):
                        un = min(TILE, tn - u)
                        cpos = loc + t0 + u
                        sl = encT_t[cpos // SLAB]
                        so = cpos % SLAB
                        nc.tensor.matmul(p[:, u:u + un], lt[:],
                                         sl[:, so: so + un],
                                         start=True, stop=True)
                    dst = e1_t[:, loc + t0: loc + t0 + tn]
                    if L1_EVAC[ei % len(L1_EVAC)] == "v":
                        nc.vector.tensor_scalar(dst, p[:, :tn], w_sb["b1"][:],
                                                0.0, op0=ADD, op1=MAX)
                    else:
                        nc.scalar.activation(dst, p[:, :tn], RELU,
                                             bias=w_sb["b1"][:], scale=1.0)
                    ei += 1
            # layer 2 with even/odd moving operands -> intra-cluster pairs
            Dbuf = stream2.tile([128, CHUNK_COLS // 2], BF16, tag="Dbuf")
            for p0 in range(0, cc, 2 * TILE):
                pc = min(2 * TILE, cc - p0)
                hn = pc // 2
                ev = e1_t[:, p0:p0 + pc].rearrange("p (i h) -> p h i", h=2)
                pAB = ps_l.tile([128, 1024], F32, tag="psl")
                nc.tensor.matmul(pAB[:, 0:hn], w_sb["fcW2"][:], ev[:, 0, :],
                                 start=True, stop=True)
                nc.tensor.matmul(pAB[:, 512:512 + hn], w_sb["fcW2"][:],
                                 ev[:, 1, :], start=True, stop=True)
                tA = small.tile([128, TILE], BF16, tag="tA2", bufs=4)
                nc.scalar.activation(tA[:, :hn], pAB[:, 0:hn], COPY)
                nc.vector.tensor_max(Dbuf[:, p0 // 2: p0 // 2 + hn],
                                     pAB[:, 512:512 + hn], tA[:, :hn])
            # stage-2 per-class direct reduce on Dbuf
            for (ri, n, w) in plan["cls_of"][k]:
                o = (int(plan["col0"][ri]) - c0) // 2
                wd = w // 2
                nc.vector.reduce_max(
                    T2buf[:, ri:ri + n],
                    Dbuf[:, o: o + n * wd].rearrange("p (n w) -> p n w", w=wd),
                    axis=mybir.AxisListType.X)

        for rep in range(reps):
            enc_tiles = {}
            M_of = {}
            enc_tiles[0] = enc_stage(0)
            M_of[0] = seg1_and_M(0)
            if n_chunks > 1:
                enc_tiles[1] = enc_stage(1)
            for k in range(n_chunks):
                if k + 2 < n_chunks:
                    enc_tiles[k + 2] = enc_stage(k + 2)
                if k + 1 < n_chunks:
                    M_of[k + 1] = seg1_and_M(k + 1)
                l_stage(k, M_of.pop(k))
                del enc_tiles[k]

            nc.scalar.activation(gT[:], T2buf[:], RELU, bias=w_sb["b2"][:],
                                 scale=1.0)

            pg = ps_l.tile([128, 1024], F32, tag="psl")
            nc.tensor.matmul(pg[:, 0:512], w_sb["G1"][:], gT[:], start=True,
                             stop=True)
            g1T = glob.tile([128, 512], BF16, tag="g1T")
            nc.scalar.activation(g1T[:], pg[:, 0:512], RELU,
                                 bias=w_sb["gb1"][:], scale=1.0)
            for half, (wn, bn) in enumerate((("G2a", "gb2a"), ("G2b", "gb2b"))):
                po = ps_l.tile([128, 1024], F32, tag="psl")
                nc.tensor.matmul(po[:, 0:512], w_sb[wn][:], g1T[:], start=True,
                                 stop=True)
                o_sb = glob.tile([128, 512], F32, tag=f"osb{half}")
                nc.scalar.activation(o_sb[:], po[:, 0:512], RELU,
                                     bias=w_sb[bn][:], scale=1.0)
                nc.sync.dma_start(out=out_d[half * 128:(half + 1) * 128, :],
                                  in_=o_sb[:])

    nc.finalize()
    return nc


# ---------------------------------------------------------------- entry

_CACHE = {}


def _run(inputs, trace=False, **spmd_kwargs):
    cluster = np.asarray(inputs["cluster"])
    key = hash(cluster.tobytes())
    if key not in _CACHE:
        plan = _plan(cluster)
        nc = _build(plan)
        _CACHE[key] = (plan, nc)
    plan, nc = _CACHE[key]

    rel_points = np.asarray(inputs["relative_points"], dtype=np.float32)
    features = np.asarray(inputs["features"], dtype=np.float32)
    sort_idx = np.argsort(cluster, kind="stable")
    bucket0 = np.concatenate(
        [[0], np.cumsum(np.bincount(cluster, minlength=N_CLUSTERS))]
    )
    wmap = _prep_weights({k: np.asarray(v, dtype=np.float32)
                          for k, v in inputs.items()
                          if k not in ("relative_points", "features", "cluster")})

    in_maps = []
    for k in range(N_CORES):
        m = _prep_core(k, plan, rel_points, features, sort_idx, bucket0)
        m.update(wmap)
        in_maps.append(m)

    res = run_bass_kernel_spmd(nc, in_maps, list(range(N_CORES)),
                               trace=trace, **spmd_kwargs)

    out = np.empty((N_CLUSTERS, 256), dtype=np.float32)
    for k in range(N_CORES):
        out[plan["cids"][k]] = res.results[k]["out"].T
    return out, res


def kernel(**inputs):
    return _run(inputs)[0]
